# revision 1
# baseline (speedup 1.0000x reference)
"""Trainium2 Bass kernel for DeformableConvBlock (B=4, C=64, H=W=128, K=3).

Self-contained: builds an SPMD Bass/Tile program for 8 NeuronCores.
Core c handles image c//2, output-row half c%2 (data-parallel over
batch x row-halves). Per core: offset conv on the PE (9 shifted
accumulating matmuls), DMA-transpose of offsets to pixel-major,
bilinear weight/index prep on the DVE, per-(tap,pixel) 2x2-patch
gathers from a precomputed channels-last patch table via the GPSIMD
SWDGE dma_gather, bilinear reduction on the DVE, DMA-transpose of the
sampled tensor to channel-major, and the 576-contraction conv matmul
on the PE.

kernel(**inputs) takes the full unsharded numpy inputs and returns the
full [4, 64, 128, 128] float32 output.
"""
from contextlib import ExitStack

import numpy as np
import ml_dtypes

import concourse.bacc as bacc
import concourse.bass as bass
import concourse.mybir as mybir
import concourse.tile as tile
from concourse.tile import TileContext
from concourse.vector_clock import ScopedClock, VectorClock

F32 = mybir.dt.float32
BF16 = mybir.dt.bfloat16
I32 = mybir.dt.int32
I16 = mybir.dt.int16
AF = mybir.ActivationFunctionType
OP = mybir.AluOpType

H = W = 128
C = 64
O = 64
KK = 9
ROWS = 64            # output rows per core
NPX = ROWS * W       # 8192
PADT = 2             # table padding on each side
PW = W + 2 * PADT    # 132
NTAB = PW * PW       # 17424 table rows
EROW = 4 * C         # 256 elems per table row
KTILES = 5           # 640 = 5*128 contraction tiles (576 used)
GROUP = 4            # rows per gather group
SUPER = 8            # rows per transpose batch
BIG = 1024.0         # floor-trick offset


class TileContextSplitDrain(TileContext):
    """Stock epilogue emits one Drain with one wait per outstanding proc;
    this walrus rejects >1 sync wait per instruction, so emit one Drain
    per proc instead."""

    def _drain_and_barrier(self, tick_clock, wait_clock):
        gc = tick_clock.global_clock
        nprocs = len(gc)
        emitted = False
        for p in range(nprocs):
            t = gc[p]
            if t <= 0:
                continue
            vec = [0] * nprocs
            vec[p] = t
            drain_inst = self.nc.sync.drain()
            wait_clock.add_sem_waits(
                drain_inst.ins, ScopedClock({None: VectorClock(vec)})
            )
            si = drain_inst.ins.sync_info
            assert si is None or len(si.on_wait) <= 1
            emitted = True
        if not emitted:
            self.nc.sync.drain()
        self.nc.all_engine_barrier()
        assert self.sems is not None
        popped = self.nc._tile_sem_poison_stack.pop()
        assert popped is self._sem_poison
        self.nc.clear_and_free_semaphores(list(self.sems.allocated().values()))
        self.nc.all_engine_barrier()


def build_program(nrows=ROWS):
    """Build the SPMD Bass program. nrows<=64 shrinks work for sim tests."""
    npx = nrows * W
    n16 = max(1, npx // 512)          # 512-px chunks for offset conv
    ngroups = nrows // GROUP
    nsupers = max(1, nrows // SUPER)

    nc = bacc.Bacc(num_swdge_queues=4)
    xband = nc.dram_tensor("xband", [C, nrows + 2, W + 2], BF16, kind="ExternalInput")
    table = nc.dram_tensor("table", [NTAB, EROW], BF16, kind="ExternalInput")
    w_off = nc.dram_tensor("w_off", [KK, C, 18], BF16, kind="ExternalInput")
    b_off = nc.dram_tensor("b_off", [18, 1], F32, kind="ExternalInput")
    w2 = nc.dram_tensor("w2", [KTILES, 128, O], BF16, kind="ExternalInput")
    b2 = nc.dram_tensor("b2", [O, 1], F32, kind="ExternalInput")
    cgrid = nc.dram_tensor("cgrid", [2, nrows * KK], F32, kind="ExternalInput")
    iotax = nc.dram_tensor("iotax", [128, 1], F32, kind="ExternalInput")
    out = nc.dram_tensor("out", [O, npx], F32, kind="ExternalOutput")

    ctx = ExitStack()
    with TileContextSplitDrain(nc) as tc:
        const_pool = ctx.enter_context(tc.tile_pool(name="const", bufs=1))
        big_pool = ctx.enter_context(tc.tile_pool(name="big", bufs=1))
        prep_pool = ctx.enter_context(tc.tile_pool(name="prep", bufs=1))
        g_pool = ctx.enter_context(tc.tile_pool(name="g", bufs=2))
        t1_pool = ctx.enter_context(tc.tile_pool(name="t1", bufs=2))
        s_pool = ctx.enter_context(tc.tile_pool(name="s", bufs=2))
        st_pool = ctx.enter_context(tc.tile_pool(name="st", bufs=2))
        o_pool = ctx.enter_context(tc.tile_pool(name="o", bufs=2))
        psum_pool = ctx.enter_context(tc.tile_pool(name="ps", bufs=2, space="PSUM"))
        psum2_pool = ctx.enter_context(tc.tile_pool(name="ps2", bufs=2, space="PSUM"))

        # ---- loads ----
        xb = const_pool.tile([C, (nrows + 2) * (W + 2)], BF16)
        nc.sync.dma_start(out=xb[:], in_=xband[:].rearrange("c h w -> c (h w)"))
        xb_v = xb[:].rearrange("c (h w) -> c h w", h=nrows + 2, w=W + 2)

        wof = const_pool.tile([C, KK * 18], BF16)
        wof_v = wof[:].rearrange("c (k e) -> c k e", k=KK, e=18)
        nc.sync.dma_start(out=wof_v, in_=w_off[:].rearrange("k c e -> c k e"))

        bof = const_pool.tile([18, 1], F32)
        nc.sync.dma_start(out=bof[:], in_=b_off[:])

        w2t = const_pool.tile([128, KTILES * O], BF16)
        w2t_v = w2t[:].rearrange("p (j e) -> p j e", j=KTILES, e=O)
        nc.sync.dma_start(out=w2t_v, in_=w2[:].rearrange("j p e -> p j e"))

        b2t = const_pool.tile([O, 1], F32)
        nc.sync.dma_start(out=b2t[:], in_=b2[:])

        # broadcast const grids across partitions
        cy = const_pool.tile([128, nrows * KK], F32)
        nc.sync.dma_start(out=cy[:], in_=cgrid[0:1, :].to_broadcast((128, nrows * KK)))
        cx = const_pool.tile([128, nrows * KK], F32)
        nc.sync.dma_start(out=cx[:], in_=cgrid[1:2, :].to_broadcast((128, nrows * KK)))
        iox = const_pool.tile([128, 1], F32)
        nc.sync.dma_start(out=iox[:], in_=iotax[:])

        # ---- stage 1: offset conv ----
        oc = big_pool.tile([32, npx], BF16)
        nc.gpsimd.memset(oc[:], 0.0)
        for t in range(n16):
            ps = psum_pool.tile([18, 512], F32, tag="ps1")
            r0 = t * 4  # first output row in chunk
            for k in range(KK):
                dy, dx = k // 3, k % 3
                rhs = xb_v[:, r0 + dy:r0 + dy + 4, dx:dx + W]
                nc.tensor.matmul(
                    out=ps[:], lhsT=wof_v[:, k, :], rhs=rhs,
                    start=(k == 0), stop=(k == KK - 1),
                )
            nc.scalar.activation(
                out=oc[:18, t * 512:(t + 1) * 512], in_=ps[:],
                func=AF.Identity, bias=bof[:],
            )

        # ---- stage 2: offsets to pixel-major ----
        op = big_pool.tile([128, nrows * 32], BF16)
        op3 = op[:].rearrange("p (y e) -> p y e", y=nrows, e=32)
        nc.sync.dma_start_transpose(out=op3, in_=oc[:])

        # ---- stage 3: wf/idx prep (all rows at once) ----
        _ppn = [0]

        def pp(dt=F32):
            _ppn[0] += 1
            return prep_pool.tile([128, nrows * KK], dt, tag=f"prep{_ppn[0]}", name=f"prep{_ppn[0]}")

        opf = prep_pool.tile([128, nrows * 18], F32, tag="opf")
        opf_v = opf[:].rearrange("p (y e) -> p y e", y=nrows, e=18)
        nc.vector.tensor_copy(out=opf_v, in_=op3[:, :, 0:18])

        wf = big_pool.tile([128, nrows * 36], BF16)
        wf_v = wf[:].rearrange("p (y k c) -> p y k c", y=nrows, k=KK, c=4)
        idx = big_pool.tile([128, nrows * KK], I16)

        for axis in (0, 1):  # 0: y, 1: x
            s = pp()
            off_src = opf_v[:, :, axis * 9:axis * 9 + 9]
            grid = cy if axis == 0 else cx
            s3 = s[:].rearrange("p (y k) -> p y k", y=nrows, k=KK)
            grid3 = grid[:].rearrange("p (y k) -> p y k", y=nrows, k=KK)
            nc.vector.tensor_tensor(out=s3, in0=off_src, in1=grid3, op=OP.add)
            if axis == 1:
                nc.vector.tensor_tensor(
                    out=s[:], in0=s[:],
                    in1=iox[:].to_broadcast((128, nrows * KK)), op=OP.add)
            # floor via +BIG truncation
            sb = pp()
            nc.vector.tensor_scalar_add(out=sb[:], in0=s[:], scalar1=BIG)
            i0 = pp(I32)
            nc.vector.tensor_copy(out=i0[:], in_=sb[:])
            f0 = pp()
            nc.vector.tensor_copy(out=f0[:], in_=i0[:])
            nc.vector.tensor_scalar_add(out=f0[:], in0=f0[:], scalar1=-BIG)
            # f0 is trunc(s)+{0,1} depending on convert rounding mode
            # (sim truncates, hw rounds-to-nearest); floor = t - (s < t)
            lt = pp()
            nc.vector.tensor_tensor(out=lt[:], in0=s[:], in1=f0[:], op=OP.is_lt)
            nc.vector.tensor_tensor(out=f0[:], in0=f0[:], in1=lt[:], op=OP.subtract)
            # fractions and validity-folded weights
            w1 = pp()
            nc.vector.tensor_tensor(out=w1[:], in0=s[:], in1=f0[:], op=OP.subtract)
            w0 = pp()
            nc.vector.tensor_scalar(
                out=w0[:], in0=w1[:], scalar1=-1.0, scalar2=1.0,
                op0=OP.mult, op1=OP.add)
            v = pp()
            vt = pp()
            nc.vector.tensor_scalar(
                out=v[:], in0=f0[:], scalar1=0.0, scalar2=127.0,
                op0=OP.is_ge, op1=OP.bypass)
            nc.vector.tensor_scalar(
                out=vt[:], in0=f0[:], scalar1=127.0, scalar2=0.0,
                op0=OP.is_le, op1=OP.bypass)
            nc.vector.tensor_tensor(out=v[:], in0=v[:], in1=vt[:], op=OP.mult)
            nc.vector.tensor_tensor(out=w0[:], in0=w0[:], in1=v[:], op=OP.mult)
            nc.vector.tensor_scalar(
                out=v[:], in0=f0[:], scalar1=-1.0, scalar2=0.0,
                op0=OP.is_ge, op1=OP.bypass)
            nc.vector.tensor_scalar(
                out=vt[:], in0=f0[:], scalar1=126.0, scalar2=0.0,
                op0=OP.is_le, op1=OP.bypass)
            nc.vector.tensor_tensor(out=v[:], in0=v[:], in1=vt[:], op=OP.mult)
            nc.vector.tensor_tensor(out=w1[:], in0=w1[:], in1=v[:], op=OP.mult)
            # clamp for index
            c0 = pp()
            nc.vector.tensor_scalar(
                out=c0[:], in0=f0[:], scalar1=-2.0, scalar2=128.0,
                op0=OP.max, op1=OP.min)
            if axis == 0:
                y_w0, y_w1, y_c0 = w0, w1, c0
            else:
                x_w0, x_w1, x_c0 = w0, w1, c0

        # wf[...,(i,j)] = wy_i * wx_j   (k-outer, corner-inner layout)
        for i, wy in enumerate((y_w0, y_w1)):
            for j, wx in enumerate((x_w0, x_w1)):
                dst = wf_v[:, :, :, 2 * i + j].rearrange("p y k -> p (y k)")
                nc.vector.tensor_tensor(out=dst, in0=wy[:], in1=wx[:], op=OP.mult)

        # idx = (yc+2)*132 + (xc+2)
        idxf = pp()
        nc.vector.tensor_scalar(
            out=idxf[:], in0=y_c0[:], scalar1=float(PW), scalar2=float(PADT * PW + PADT),
            op0=OP.mult, op1=OP.add)
        nc.vector.tensor_tensor(out=idxf[:], in0=idxf[:], in1=x_c0[:], op=OP.add)
        nc.vector.tensor_copy(out=idx[:], in_=idxf[:])
        # wrapped-16 index layout for dma_gather queue 0 (cores 0/1 read
        # partitions 0-15 / 16-31): wrapped[r*16+p16, yk*8+q] = idx[q*16+p16, yk]
        # for r in {0,1}. Built with partition stream-shuffles, no DMA.
        nyk = nrows * KK
        wrapped = big_pool.tile([128, nyk * 8], I16)
        nc.gpsimd.memset(wrapped[:], 0)
        wr_v = wrapped[:].rearrange("p (yk q) -> p yk q", yk=nyk, q=8)
        for qj in range(4):
            for qh in range(2):
                mask = [16 * qh + (p % 16) for p in range(32)]
                nc.vector.stream_shuffle(
                    out=wr_v[0:32, :, 2 * qj + qh],
                    in_=idx[32 * qj:32 * (qj + 1), :], mask=mask)
        for qt in range(1, 4):
            nc.vector.tensor_copy(
                out=wrapped[32 * qt:32 * (qt + 1), :], in_=wrapped[0:32, :])

        # ---- stages 4-6 ----
        for sg in range(nsupers):
            s8 = s_pool.tile([128, SUPER * KTILES * 128], BF16, tag="s8")
            s8_v = s8[:].rearrange("p (y e) -> p y e", y=SUPER, e=KTILES * 128)
            nc.vector.memset(s8_v[:, :, KK * C:], 0.0)
            for gi in range(SUPER // GROUP):
                y0 = sg * SUPER + gi * GROUP
                g = g_pool.tile([128, GROUP * KK * EROW], BF16, tag="g")
                g_m = g[:].rearrange("p (m e) -> p m e", m=GROUP * KK, e=EROW)
                nidx_g = 128 * GROUP * KK
                nc.gpsimd.dma_gather(
                    out_ap=g_m, in_ap=table[:],
                    idxs_ap=wrapped[:, y0 * KK * 8:(y0 + GROUP) * KK * 8],
                    num_idxs=nidx_g, num_idxs_reg=nidx_g, elem_size=EROW,
                    single_packet=False, queue_num=(y0 // GROUP) % 4)
                g_v = g[:].rearrange(
                    "p (y k c e) -> p y k c e", y=GROUP, k=KK, c=C, e=4)
                wfb = wf_v[:, y0:y0 + GROUP, :, None, :].to_broadcast(
                    (128, GROUP, KK, C, 4))
                nc.vector.tensor_tensor(out=g_v, in0=g_v, in1=wfb, op=OP.mult)
                t1 = t1_pool.tile([128, GROUP * KK * C * 2], BF16, tag="t1")
                t1_v = t1[:].rearrange(
                    "p (y k c e) -> p y k c e", y=GROUP, k=KK, c=C, e=2)
                nc.vector.tensor_tensor(
                    out=t1_v, in0=g_v[:, :, :, :, 0:2], in1=g_v[:, :, :, :, 2:4],
                    op=OP.add)
                sdst = s8_v[:, gi * GROUP:(gi + 1) * GROUP, 0:KK * C].rearrange(
                    "p y (k c) -> p y k c", k=KK, c=C)
                nc.vector.tensor_tensor(
                    out=sdst, in0=t1_v[:, :, :, :, 0], in1=t1_v[:, :, :, :, 1],
                    op=OP.add)

            st = st_pool.tile([128, SUPER * KTILES * 128], BF16, tag="st")
            st_v = st[:].rearrange("p (m c) -> p m c", m=SUPER * KTILES, c=128)
            nc.sync.dma_start_transpose(out=st_v, in_=s8[:])
            st_y = st[:].rearrange("p (y j c) -> p y j c", y=SUPER, j=KTILES, c=128)

            for half in range(SUPER * 128 // 512):
                ps2 = psum2_pool.tile([O, 512], F32, tag="ps2")
                for j in range(KTILES):
                    rhs = st_y[:, 4 * half:4 * half + 4, j, :]
                    nc.tensor.matmul(
                        out=ps2[:], lhsT=w2t_v[:, j, :], rhs=rhs,
                        start=(j == 0), stop=(j == KTILES - 1),
                    )
                ob = o_pool.tile([O, 512], F32, tag="ob")
                nc.scalar.activation(
                    out=ob[:], in_=ps2[:], func=AF.Identity, bias=b2t[:])
                pc = sg * (SUPER * 128 // 512) + half
                nc.sync.dma_start(out=out[:, pc * 512:(pc + 1) * 512], in_=ob[:])
        ctx.close()
    nc.compile()
    return nc


# ---------------- host side ----------------

def make_constants():
    """Per-core constant tensors (identical across cores except cgrid)."""
    ky, kx = np.meshgrid(np.arange(3), np.arange(3), indexing="ij")
    ky = ky.reshape(KK).astype(np.float32)
    kx = kx.reshape(KK).astype(np.float32)
    return ky, kx


def host_prepare(x, off_w, off_b, weight, bias, nrows=ROWS):
    """Build per-core input maps."""
    B = x.shape[0]
    x = np.asarray(x, np.float32)
    # padded image for offset conv, bf16, [B, C, H+2, W+2]
    xpad = np.pad(x, ((0, 0), (0, 0), (1, 1), (1, 1))).astype(ml_dtypes.bfloat16)
    # patch table per image: padded-by-2 channels-last
    xp2 = np.pad(x, ((0, 0), (0, 0), (PADT, PADT + 1), (PADT, PADT + 1)))
    # [B, PW+1, PW+1, C] channels-last
    xcl = xp2.transpose(0, 2, 3, 1)
    # table[q=(yp*PW+xp)] = interleave over c: [x(y,x,c), x(y,x+1,c), x(y+1,x,c), x(y+1,x+1,c)]
    tables = []
    for b in range(B):
        t = np.empty((PW, PW, C, 4), np.float32)
        t[:, :, :, 0] = xcl[b, :PW, :PW]
        t[:, :, :, 1] = xcl[b, :PW, 1:PW + 1]
        t[:, :, :, 2] = xcl[b, 1:PW + 1, :PW]
        t[:, :, :, 3] = xcl[b, 1:PW + 1, 1:PW + 1]
        tables.append(t.reshape(NTAB, EROW).astype(ml_dtypes.bfloat16))

    # offset conv weights: channel perm [dy taps 0..8, dx taps 0..8]
    perm = [2 * k for k in range(KK)] + [2 * k + 1 for k in range(KK)]
    w_off_p = np.asarray(off_w, np.float32)[perm]          # [18, C, 3, 3]
    # lhsT per tap: tap k = dy*3+dx -> [C, 18]
    w_off_t = np.empty((KK, C, 18), np.float32)
    for k in range(KK):
        dy, dx = k // 3, k % 3
        w_off_t[k] = w_off_p[:, :, dy, dx].T               # [C, 18]
    w_off_t = w_off_t.astype(ml_dtypes.bfloat16)
    b_off_p = np.asarray(off_b, np.float32)[perm].reshape(18, 1)

    # main weights: W2[(k,c), o] = weight[o, c, k], padded to 640 rows
    wgt = np.asarray(weight, np.float32).reshape(O, C, KK)
    w2f = np.zeros((KTILES * 128, O), np.float32)
    kc = wgt.transpose(2, 1, 0).reshape(KK * C, O)          # [(k,c), O]
    w2f[:KK * C] = kc
    w2f = w2f.reshape(KTILES, 128, O).astype(ml_dtypes.bfloat16)
    b2f = np.asarray(bias, np.float32).reshape(O, 1)

    ky, kx = make_constants()
    iotax = np.arange(128, dtype=np.float32).reshape(128, 1)

    in_maps = []
    for core in range(8):
        b, hh = core // 2, core % 2
        y0 = hh * 64
        rows = np.arange(y0, y0 + nrows, dtype=np.float32)
        cgy = (rows[:, None] + ky[None, :] - 1.0).reshape(1, nrows * KK)
        cgx = np.broadcast_to(kx[None, :] - 1.0, (nrows, KK)).reshape(1, nrows * KK)
        cgrid = np.concatenate([cgy, cgx], 0).astype(np.float32)
        in_maps.append({
            "xband": np.ascontiguousarray(xpad[b, :, y0:y0 + nrows + 2, :]),
            "table": tables[b],
            "w_off": w_off_t,
            "b_off": b_off_p,
            "w2": w2f,
            "b2": b2f,
            "cgrid": cgrid,
            "iotax": iotax,
        })
    return in_maps


def assemble(outs, nrows=ROWS):
    """outs: list of 8 dicts with 'out' [O, nrows*W] -> [4, O, H, W]"""
    full = np.zeros((4, O, H, W), np.float32)
    for core, om in enumerate(outs):
        b, hh = core // 2, core % 2
        full[b, :, hh * 64:hh * 64 + nrows] = om["out"].reshape(O, nrows, W)
    return full


_CACHE = {}


def kernel(x, off_w, off_b, weight, bias):
    if "nc" not in _CACHE:
        _CACHE["nc"] = build_program()
    nc = _CACHE["nc"]
    in_maps = host_prepare(x, off_w, off_b, weight, bias)
    from concourse.bass_utils import run_bass_kernel_spmd
    res = run_bass_kernel_spmd(nc, in_maps, core_ids=list(range(8)))
    return assemble(res.results)

